# revision 1
# baseline (speedup 1.0000x reference)
"""Trainium2 Bass kernel for the labelled contrastive loss.

Math (per batch row b, label L, over C=200 centers):
    cos[b,c] = <f_b, c_c> / (|f_b| |c_c|)
    a = |cos|;  l1_b = sum_c a[b,c];  row term = (2*a[b,L_b] - l1_b)/l1_b
    loss = -sum over labelled rows of row term
The feature norm |f_b| cancels in the ratio, so the kernel never computes
it: it works on raw = f @ cn^T with cn = centers/max(|c|,eps) normalized on
host (O(C*D), negligible), and forms (2*T - S)/S with
    S = sum_c |raw|,  T = |raw[b, L_b]|.

Sharding: data-parallel over the batch axis, 4096 rows per core across
8 cores; centers replicated. Per-core output is a [128,1] vector of
per-partition partial sums; the host adds them up and negates.

Device pipeline, two 128-row tiles ("a pair") at a time:
    DMA   : feature chunks [128d x 2 x 6 x 128b] (host pre-transposed so the
            contraction dim is on partitions -- no on-chip transposes)
    PE    : 2x6 accumulating matmuls (bf16 in, f32 PSUM) -> cos pair
            [128b, 2, 200c] in a single PSUM bank
    ACT   : per tile, Abs with accum_out -> exact f32 S column (the |cos|
            output itself is a throwaway; only the accumulator is used)
    DVE   : one-hot mask = is_equal(iota, label broadcast); signed
            T = rowsum(cos * mask), batched over the pair; f32 throughout
Epilogue on [128, 32] f32 tiles: T=|T|; msk * (2T - S)/S; row-reduce; DMA.

bf16 is used only for the matmul inputs; S is accumulated in f32 from the
f32 PSUM and the final ratio is f32, so input rounding enters the per-row
term only at second order (measured ~1e-7 relative on the final scalar).
"""

import numpy as np
import ml_dtypes

import concourse.bass as bass
import concourse.tile as tile
from concourse import mybir
from concourse.bass_utils import run_bass_kernel_spmd

# ---------------------------------------------------------------------------
# Workaround for walrus "Too many sync wait commands": this toolchain only
# encodes a limited number of sem waits per instruction, so spread excess
# waits over preceding same-engine nops — both for scheduled instructions
# (pre-lowering pass) and for the TileContext tail drain.
# ---------------------------------------------------------------------------
from concourse.vector_clock import ScopedClock

_MAX_WAITS = 1
_split_counter = [0]


def _split_waits_in_ordered(ordered):
    for bb_name, insts in ordered.items():
        new = []
        for inst in insts:
            si = getattr(inst, "sync_info", None)
            waits = list(si.on_wait) if si is not None and si.on_wait else []
            if len(waits) > _MAX_WAITS:
                updates = list(si.on_update) if si.on_update else []
                head, tail = waits[:-_MAX_WAITS], waits[-_MAX_WAITS:]
                while head:
                    n = mybir.InstNoOp(
                        name=f"I-wsplit-{_split_counter[0]}", ins=[], outs=[]
                    )
                    _split_counter[0] += 1
                    n.engine = inst.engine
                    n.bass_nofuse = True
                    n.sync_info = mybir.SyncInfo(
                        on_wait=head[:_MAX_WAITS], on_update=[]
                    )
                    head = head[_MAX_WAITS:]
                    new.append(n)
                inst.sync_info = mybir.SyncInfo(on_wait=tail, on_update=updates)
            new.append(inst)
        ordered[bb_name] = new


_orig_lower_ordered = tile.TileContext._lower_ordered_insts


def _patched_lower_ordered(self, ordered):
    _split_waits_in_ordered(ordered)
    return _orig_lower_ordered(self, ordered)


tile.TileContext._lower_ordered_insts = _patched_lower_ordered


def _patched_drain_and_barrier(self, tick_clock, wait_clock):
    """Minimal kernel tail replacing the stock drain + two EVSEM-butterfly
    barriers (~15us):

    1. SP nops carry one sem wait each for every proc's final clock tick —
       once they pass, every tracked semaphore increment has LANDED (waits
       observe the final value of each proc's latest sem; same-engine and
       same-queue increments retire in order).
    2. Each engine drains its pipeline and bumps a tail semaphore; once it
       passes its own last wait nothing can block it, so this retires.
    3. GpSimd waits for the 4 other engines + SP, then range-clears all
       tile semaphores, resets DMA queue state and clears the tail sem.
    4. Engines halt independently; the NEFF only completes (and can only
       be re-executed) when every engine including GpSimd has halted, so
       the next run starts with everything zeroed.
    """
    nc = self.nc
    carrier = nc.sync.nop(nofuse=True)
    wait_clock.add_sem_waits(carrier.ins, ScopedClock({None: tick_clock.global_clock}))
    si = carrier.ins.sync_info
    waits = list(si.on_wait) if si is not None and si.on_wait else []
    if len(waits) > _MAX_WAITS:
        updates = list(si.on_update) if si.on_update else []
        carrier.ins.sync_info = mybir.SyncInfo(on_wait=[], on_update=updates)
        rest = waits
        while rest:
            n = nc.sync.nop(nofuse=True)
            n.ins.sync_info = mybir.SyncInfo(on_wait=rest[:_MAX_WAITS], on_update=[])
            rest = rest[_MAX_WAITS:]
    nc.sync.drain()

    tail_sem = nc.alloc_semaphore("tile_tail_sem")
    n_inc = 0
    for eng_type, eng in nc.engines.items():
        if eng_type == mybir.EngineType.Pool:
            continue
        eng.drain()
        eng.sem_inc(tail_sem, 1)
        n_inc += 1
    nc.gpsimd.drain()
    nc.gpsimd.wait_ge(tail_sem, n_inc)

    assert self.sems is not None
    popped = nc._tile_sem_poison_stack.pop()
    assert popped is self._sem_poison
    nc.clear_and_free_semaphores(list(self.sems.allocated().values()))
    nc.clear_and_free_semaphores([tail_sem])


tile.TileContext._drain_and_barrier = _patched_drain_and_barrier

# ---------------------------------------------------------------------------
# Problem constants (hardcoded per contract)
# ---------------------------------------------------------------------------
N_CORES = 8
B, D, C = 32768, 768, 200
B_CORE = B // N_CORES          # 4096
P = 128                        # partitions
KCH = D // P                   # 6 contraction chunks
NT = B_CORE // P               # 32 tiles per core
NPAIR = NT // 2                # 16 pairs
CP = 256                       # padded per-tile PSUM pitch (bank alignment)
EPS_COS = 1e-8

_TRACE = False                 # test.py flips this for profiling runs
_TRACE_DIR = None
last_results = None

_nc = None


def _build():
    global _nc
    if _nc is not None:
        return _nc
    nc = bass.Bass("TRN2", debug=False, num_devices=N_CORES)

    bf16 = mybir.dt.bfloat16
    f32 = mybir.dt.float32

    # ft[quad, p, t', k, b] = features[(4*quad+t')*128 + b, k*128 + p], bf16
    ft = nc.dram_tensor("ft", [NPAIR, P, 2, KCH, P], bf16, kind="ExternalInput")
    cnt = nc.dram_tensor("cnt", [P, KCH, C], bf16, kind="ExternalInput")
    iota = nc.dram_tensor("iota", [P, 4, C], f32, kind="ExternalInput")
    lab = nc.dram_tensor("lab", [P, NT], f32, kind="ExternalInput")
    msk = nc.dram_tensor("msk", [P, NT], f32, kind="ExternalInput")
    out = nc.dram_tensor("out", [1, 1], f32, kind="ExternalOutput")

    with tile.TileContext(nc) as tc:
        with (
            tc.tile_pool(name="singles", bufs=1) as singles,
            tc.tile_pool(name="ftp", bufs=8) as ftp,
            tc.tile_pool(name="maskp", bufs=8) as maskp,
            tc.tile_pool(name="work", bufs=4) as work,
            tc.tile_pool(name="psum", bufs=4, space="PSUM") as psum,
            tc.tile_pool(name="psum1", bufs=1, space="PSUM") as psum1,
        ):
            cnt_sb = singles.tile([P, KCH, C], bf16)
            nc.sync.dma_start(cnt_sb[:], cnt[:])
            iota_sb = singles.tile([P, 4, C], f32)
            nc.sync.dma_start(iota_sb[:], iota[:])
            lab_sb = singles.tile([P, NT], f32)
            nc.sync.dma_start(lab_sb[:], lab[:])
            msk_sb = singles.tile([P, NT], f32)
            nc.sync.dma_start(msk_sb[:], msk[:])

            s_all = singles.tile([P, NT], f32)
            t_all = singles.tile([P, NT], f32)

            for pr in range(NPAIR):
                t0 = 2 * pr
                ft_sb = ftp.tile([P, 2, KCH, P], bf16)
                nc.sync.dma_start(ft_sb[:], ft[pr])

                # one-hot masks for 4 tiles at a time (2 pairs)
                if pr % 2 == 0:
                    mask_sb = maskp.tile([P, 4, C], f32, tag="mask")
                    nc.vector.tensor_tensor(
                        out=mask_sb[:],
                        in0=iota_sb[:],
                        in1=lab_sb[:, t0 : t0 + 4].broadcast_to([P, 4, C]),
                        op=mybir.AluOpType.is_equal,
                    )
                mhalf = (pr % 2) * 2

                cos_ps = psum.tile([P, 2, C], f32)
                for j in range(2):
                    for k in range(KCH):
                        nc.tensor.matmul(
                            cos_ps[:, j, :],
                            ft_sb[:, j, k, :],
                            cnt_sb[:, k, :],
                            start=(k == 0),
                            stop=(k == KCH - 1),
                        )

                # S columns: ACT Abs with row-sum accumulator (out is junk)
                junk_sb = work.tile([P, 2, C], bf16, tag="junk")
                for j in range(2):
                    nc.scalar.activation(
                        out=junk_sb[:, j, :],
                        in_=cos_ps[:, j, :],
                        func=mybir.ActivationFunctionType.Abs,
                        accum_out=s_all[:, t0 + j : t0 + j + 1],
                    )

                # signed T columns for the pair on DVE (f32)
                am_sb = work.tile([P, 2, C], f32, tag="am")
                nc.vector.tensor_tensor(
                    out=am_sb[:], in0=cos_ps[:],
                    in1=mask_sb[:, mhalf : mhalf + 2, :],
                    op=mybir.AluOpType.mult,
                )
                nc.vector.tensor_reduce(
                    out=t_all[:, t0 : t0 + 2], in_=am_sb[:],
                    op=mybir.AluOpType.add, axis=mybir.AxisListType.X,
                )

            # epilogue: T = |T|; per-row term = msk * (2*T - S) / S; reduce
            t_abs = singles.tile([P, NT], f32)
            nc.scalar.activation(
                out=t_abs[:], in_=t_all[:],
                func=mybir.ActivationFunctionType.Abs,
            )
            recip = singles.tile([P, NT], f32)
            nc.vector.reciprocal(recip[:], s_all[:])
            num = singles.tile([P, NT], f32)
            nc.vector.tensor_scalar(
                out=num[:],
                in0=t_abs[:],
                scalar1=2.0,
                scalar2=None,
                op0=mybir.AluOpType.mult,
            )
            nc.vector.tensor_tensor(
                out=num[:], in0=num[:], in1=s_all[:], op=mybir.AluOpType.subtract
            )
            nc.vector.tensor_tensor(
                out=num[:], in0=num[:], in1=recip[:], op=mybir.AluOpType.mult
            )
            nc.vector.tensor_tensor(
                out=num[:], in0=num[:], in1=msk_sb[:], op=mybir.AluOpType.mult
            )
            # collapse to one scalar on-chip: PE sums over partitions, DVE
            # over the NT columns -- so the store is a single 4B descriptor
            # (a [128,1] store would spray 128 tiny descriptors over all 16
            # DMA engines, whose completion events straggle for ~6us).
            ones_sb = singles.tile([P, 1], f32)
            nc.vector.memset(ones_sb[:], 1.0)
            tot_ps = psum1.tile([1, NT], f32)
            nc.tensor.matmul(tot_ps[:], ones_sb[:], num[:], start=True, stop=True)
            out_sb = singles.tile([1, 1], f32)
            nc.vector.tensor_reduce(
                out=out_sb[:], in_=tot_ps[:], op=mybir.AluOpType.add,
                axis=mybir.AxisListType.X,
            )
            nc.sync.dma_start(out[:], out_sb[:])

    _nc = nc
    return nc


def kernel(features, centers, labels, labelled_or_not):
    global last_results
    nc = _build()

    bf = ml_dtypes.bfloat16
    features = np.asarray(features, dtype=np.float32)
    centers = np.asarray(centers, dtype=np.float32)
    labels_f = np.asarray(labels).astype(np.float32)
    msk_f = np.asarray(labelled_or_not).astype(np.float32)

    # normalized + transposed centers -> [P, KCH, C] in bf16
    cn = centers / np.maximum(
        np.linalg.norm(centers, axis=1, keepdims=True), EPS_COS
    )
    cnt_host = np.ascontiguousarray(
        cn.reshape(C, KCH, P).transpose(2, 1, 0).astype(bf)
    )
    iota_host = np.ascontiguousarray(
        np.broadcast_to(np.arange(C, dtype=np.float32), (P, 4, C))
    )

    in_maps = []
    for c in range(N_CORES):
        sl = slice(c * B_CORE, (c + 1) * B_CORE)
        fcore = features[sl]  # [4096, 768]
        # ft[pair, p, t', k, b] = f[(2*pair+t')*128 + b, k*128 + p]
        ft_host = np.ascontiguousarray(
            fcore.reshape(NPAIR, 2, P, KCH, P).transpose(0, 4, 1, 3, 2).astype(bf)
        )
        lab_host = np.ascontiguousarray(labels_f[sl].reshape(NT, P).T)
        msk_host = np.ascontiguousarray(msk_f[sl].reshape(NT, P).T)
        in_maps.append(
            {
                "ft": ft_host,
                "cnt": cnt_host,
                "iota": iota_host,
                "lab": lab_host,
                "msk": msk_host,
            }
        )

    kwargs = {}
    if _TRACE:
        kwargs["trace"] = True
        if _TRACE_DIR:
            kwargs["tmpdir"] = _TRACE_DIR
    res = run_bass_kernel_spmd(nc, in_maps, core_ids=list(range(N_CORES)), **kwargs)
    last_results = res

    total = 0.0
    for c in range(N_CORES):
        total += float(res.results[c]["out"][0, 0])
    return np.array(-total, dtype=np.float32)



# revision 4
# speedup vs baseline: 1.4522x; 1.4522x over previous
"""Trainium2 Bass kernel for the labelled contrastive loss.

Math (per batch row b, label L, over C=200 centers):
    cos[b,c] = <f_b, c_c> / (|f_b| |c_c|)
    a = |cos|;  S_b = sum_c a[b,c];  T_b = a[b,L_b]
    row term = (2*T - S)/S;  loss = -sum over labelled rows of row term
The feature norm |f_b| cancels in the ratio, so the kernel works on
raw = f @ cn^T with cn = 16*centers/max(|c|,eps) normalized+scaled on host
(the x16 keeps fp8 center values in the normal range; the ratio is
scale-invariant).

Key structural choices vs a straight data-parallel port:
  * Only LABELLED rows are shipped to the device (host compaction): the
    unlabelled ~half of the batch contributes nothing.  Rows are padded
    to a static 18432 (2304 per core = 9 pairs of 128-row tiles) with a
    ones-feature row and msk=0 (S>0 so no NaN; msk kills the term).
  * fp8(e4m3) matmul inputs with DoubleRow perf mode: contraction 256
    per matmul instruction -> 6 matmuls per 256-row pair (vs 24 bf16).
  * Per pair, ACT does one Abs activation PSUM->SBUF bf16 (A = |cos|).
    Per tile, DVE does two fused ops only:
      S: tensor_scalar(copy, accum_out)  on A   (4x mode, bf16 SBUF)
      T: scalar_tensor_tensor((iota==lab)*A, accum_out)  -- builds the
         one-hot mask, applies it and row-reduces in ONE instruction.
    No mask tensors, no separate reduces, no accumulator readouts.
  * Centers are zero-padded to 208 columns (DoubleRow needs the k-tile
    stride %16==0); padded cos columns are exactly 0 so S and T are
    unaffected.
  * ft is DMAed in 3 chunks of 3 pairs with a partition-contiguous DRAM
    layout (one 4608B run per partition per chunk).

Epilogue on [128, 18] f32 tiles: term = msk * (2T - S)/S; PE collapses
partitions via a ones-matmul, DVE reduces the 18 columns, single 4B
store.  Host adds the 8 per-core scalars and negates.
"""

import numpy as np
import ml_dtypes

import concourse.bass as bass
import concourse.tile as tile
from concourse import mybir
from concourse.bass_utils import run_bass_kernel_spmd

# ---------------------------------------------------------------------------
# Workaround for walrus "Too many sync wait commands": this toolchain only
# encodes a limited number of sem waits per instruction, so spread excess
# waits over preceding same-engine nops — both for scheduled instructions
# (pre-lowering pass) and for the TileContext tail drain.
# ---------------------------------------------------------------------------
from concourse.vector_clock import ScopedClock

_MAX_WAITS = 1
_split_counter = [0]


def _split_waits_in_ordered(ordered):
    for bb_name, insts in ordered.items():
        new = []
        for inst in insts:
            si = getattr(inst, "sync_info", None)
            waits = list(si.on_wait) if si is not None and si.on_wait else []
            if len(waits) > _MAX_WAITS:
                updates = list(si.on_update) if si.on_update else []
                head, tail = waits[:-_MAX_WAITS], waits[-_MAX_WAITS:]
                while head:
                    n = mybir.InstNoOp(
                        name=f"I-wsplit-{_split_counter[0]}", ins=[], outs=[]
                    )
                    _split_counter[0] += 1
                    n.engine = inst.engine
                    n.bass_nofuse = True
                    n.sync_info = mybir.SyncInfo(
                        on_wait=head[:_MAX_WAITS], on_update=[]
                    )
                    head = head[_MAX_WAITS:]
                    new.append(n)
                inst.sync_info = mybir.SyncInfo(on_wait=tail, on_update=updates)
            new.append(inst)
        ordered[bb_name] = new


_orig_lower_ordered = tile.TileContext._lower_ordered_insts


def _patched_lower_ordered(self, ordered):
    _split_waits_in_ordered(ordered)
    return _orig_lower_ordered(self, ordered)


tile.TileContext._lower_ordered_insts = _patched_lower_ordered


def _patched_drain_and_barrier(self, tick_clock, wait_clock):
    """Minimal kernel tail replacing the stock drain + two EVSEM-butterfly
    barriers (~15us):

    1. SP nops carry one sem wait each for every proc's final clock tick —
       once they pass, every tracked semaphore increment has LANDED (waits
       observe the final value of each proc's latest sem; same-engine and
       same-queue increments retire in order).
    2. Each engine drains its pipeline and bumps a tail semaphore; once it
       passes its own last wait nothing can block it, so this retires.
    3. GpSimd waits for the 4 other engines + SP, then range-clears all
       tile semaphores, resets DMA queue state and clears the tail sem.
    4. Engines halt independently; the NEFF only completes (and can only
       be re-executed) when every engine including GpSimd has halted, so
       the next run starts with everything zeroed.
    """
    nc = self.nc
    carrier = nc.sync.nop(nofuse=True)
    wait_clock.add_sem_waits(carrier.ins, ScopedClock({None: tick_clock.global_clock}))
    si = carrier.ins.sync_info
    waits = list(si.on_wait) if si is not None and si.on_wait else []
    if len(waits) > _MAX_WAITS:
        updates = list(si.on_update) if si.on_update else []
        carrier.ins.sync_info = mybir.SyncInfo(on_wait=[], on_update=updates)
        rest = waits
        while rest:
            n = nc.sync.nop(nofuse=True)
            n.ins.sync_info = mybir.SyncInfo(on_wait=rest[:_MAX_WAITS], on_update=[])
            rest = rest[_MAX_WAITS:]
    nc.sync.drain()

    tail_sem = nc.alloc_semaphore("tile_tail_sem")
    n_inc = 0
    for eng_type, eng in nc.engines.items():
        if eng_type == mybir.EngineType.Pool:
            continue
        eng.drain()
        eng.sem_inc(tail_sem, 1)
        n_inc += 1
    nc.gpsimd.drain()
    nc.gpsimd.wait_ge(tail_sem, n_inc)

    assert self.sems is not None
    popped = nc._tile_sem_poison_stack.pop()
    assert popped is self._sem_poison
    nc.clear_and_free_semaphores(list(self.sems.allocated().values()))
    nc.clear_and_free_semaphores([tail_sem])


tile.TileContext._drain_and_barrier = _patched_drain_and_barrier

# ---------------------------------------------------------------------------
# Problem constants (hardcoded per contract)
# ---------------------------------------------------------------------------
N_CORES = 8
B, D, C = 32768, 768, 200
CPAD = 208                     # padded center count (DoubleRow stride %16)
KS = D // 128                  # 6 contraction subtiles of 128
B_CAP = 18432                  # static labelled-row capacity per device pass
B_CORE = B_CAP // N_CORES      # 2304 rows per core
NT = B_CORE // 128             # 18 tiles per core
NPAIR = NT // 2                # 9 pairs
FT_CHUNKS = 3                  # ft DMA granularity: 3 pairs per chunk
PAIRS_PER_CHUNK = NPAIR // FT_CHUNKS
EPS_COS = 1e-8
CSCALE = 16.0                  # power-of-2 center scale; cancels in the ratio

_TRACE = False                 # test.py flips this for profiling runs
_TRACE_DIR = None
last_results = None

_nc = None


def _build():
    global _nc
    if _nc is not None:
        return _nc
    nc = bass.Bass("TRN2", debug=False, num_devices=N_CORES)

    f8 = mybir.dt.float8e4
    bf16 = mybir.dt.bfloat16
    f32 = mybir.dt.float32

    # ft[ch, p, w, j, ks, b] = f[((3*ch+w)*2+j)*128 + b, ks*128 + p], fp8
    ft = nc.dram_tensor(
        "ft", [FT_CHUNKS, 128, PAIRS_PER_CHUNK, 2, KS, 128], f8,
        kind="ExternalInput",
    )
    cnt = nc.dram_tensor("cnt", [128, KS, CPAD], f8, kind="ExternalInput")
    iota = nc.dram_tensor("iota", [128, CPAD], bf16, kind="ExternalInput")
    lab = nc.dram_tensor("lab", [128, NT], bf16, kind="ExternalInput")
    msk = nc.dram_tensor("msk", [128, NT], f32, kind="ExternalInput")
    out = nc.dram_tensor("out", [1, 1], f32, kind="ExternalOutput")

    with tile.TileContext(nc) as tc:
        with (
            tc.tile_pool(name="singles", bufs=1) as singles,
            tc.tile_pool(name="ftp", bufs=FT_CHUNKS) as ftp,
            tc.tile_pool(name="apool", bufs=4) as apool,
            tc.tile_pool(name="junk", bufs=4) as junk,
            tc.tile_pool(name="psum", bufs=4, space="PSUM") as psum,
            tc.tile_pool(name="psum1", bufs=1, space="PSUM") as psum1,
        ):
            ft_sb = []
            for ch in range(FT_CHUNKS):
                t = ftp.tile([128, PAIRS_PER_CHUNK, 2, KS, 128], f8, tag="ft")
                ft_sb.append(t)
            # first feature chunk starts the wire immediately; the small
            # constants queue right behind it, then the remaining chunks
            nc.sync.dma_start(ft_sb[0][:], ft[0])
            cnt_sb = singles.tile([128, KS, CPAD], f8)
            nc.sync.dma_start(cnt_sb[:], cnt[:])
            iota_sb = singles.tile([128, CPAD], bf16)
            nc.sync.dma_start(iota_sb[:], iota[:])
            lab_sb = singles.tile([128, NT], bf16)
            nc.sync.dma_start(lab_sb[:], lab[:])
            msk_sb = singles.tile([128, NT], f32)
            nc.sync.dma_start(msk_sb[:], msk[:])
            for ch in range(1, FT_CHUNKS):
                nc.sync.dma_start(ft_sb[ch][:], ft[ch])

            s_all = singles.tile([128, NT], f32)
            t_all = singles.tile([128, NT], f32)

            for pr in range(NPAIR):
                ch, wi = divmod(pr, PAIRS_PER_CHUNK)
                fts = ft_sb[ch]

                cos = psum.tile([128, 2, CPAD], f32)
                for j in range(2):
                    for kk in range(KS // 2):
                        nc.tensor.matmul(
                            cos[:, j, :],
                            fts[:, wi, j, 2 * kk : 2 * kk + 2, :],
                            cnt_sb[:, 2 * kk : 2 * kk + 2, :],
                            start=(kk == 0),
                            stop=(kk == KS // 2 - 1),
                            perf_mode=mybir.MatmulPerfMode.DoubleRow,
                        )

                # A = |cos| in bf16 (one ACT pass per pair, no accum)
                a_sb = apool.tile([128, 2, CPAD], bf16, tag="abs")
                nc.scalar.activation(
                    out=a_sb[:], in_=cos[:],
                    func=mybir.ActivationFunctionType.Abs,
                )

                for j in range(2):
                    t = 2 * pr + j
                    # S column: single-src copy with sum-accumulator (4x)
                    js = junk.tile([128, CPAD], bf16, tag="js")
                    nc.vector.tensor_scalar(
                        out=js[:], in0=a_sb[:, j, :],
                        scalar1=0.0, scalar2=0.0,
                        op0=mybir.AluOpType.add,
                        op1=mybir.AluOpType.add,
                        accum_out=s_all[:, t : t + 1],
                    )
                    # T column: (iota == lab)*A summed, all in one op
                    jt = junk.tile([128, CPAD], bf16, tag="jt")
                    nc.vector.scalar_tensor_tensor(
                        out=jt[:], in0=iota_sb[:],
                        scalar=lab_sb[:, t : t + 1],
                        in1=a_sb[:, j, :],
                        op0=mybir.AluOpType.is_equal,
                        op1=mybir.AluOpType.mult,
                        accum_out=t_all[:, t : t + 1],
                    )

            # epilogue: term = msk * (2*T - S) / S; collapse to one scalar
            recip = singles.tile([128, NT], f32)
            nc.vector.reciprocal(recip[:], s_all[:])
            num = singles.tile([128, NT], f32)
            nc.vector.tensor_scalar(
                out=num[:], in0=t_all[:],
                scalar1=2.0, scalar2=None,
                op0=mybir.AluOpType.mult,
            )
            nc.vector.tensor_tensor(
                out=num[:], in0=num[:], in1=s_all[:],
                op=mybir.AluOpType.subtract,
            )
            nc.vector.tensor_tensor(
                out=num[:], in0=num[:], in1=recip[:], op=mybir.AluOpType.mult
            )
            nc.vector.tensor_tensor(
                out=num[:], in0=num[:], in1=msk_sb[:], op=mybir.AluOpType.mult
            )
            # PE sums over partitions, DVE over the NT columns, so the store
            # is a single 4B descriptor (a [128,1] store would spray 128
            # tiny descriptors over all 16 DMA engines).
            ones_sb = singles.tile([128, 1], f32)
            nc.vector.memset(ones_sb[:], 1.0)
            tot_ps = psum1.tile([1, NT], f32)
            nc.tensor.matmul(tot_ps[:], ones_sb[:], num[:], start=True, stop=True)
            out_sb = singles.tile([1, 1], f32)
            nc.vector.tensor_reduce(
                out=out_sb[:], in_=tot_ps[:], op=mybir.AluOpType.add,
                axis=mybir.AxisListType.X,
            )
            nc.sync.dma_start(out[:], out_sb[:])

    _nc = nc
    return nc


def _run_pass(feats_sel, labs_sel, msk_sel, cnt_host, iota_host):
    """One device pass over up to B_CAP compacted rows (padded)."""
    global last_results
    nc = _build()
    f8np = ml_dtypes.float8_e4m3
    bfnp = ml_dtypes.bfloat16

    n = feats_sel.shape[0]
    fpad = np.ones((B_CAP, D), dtype=np.float32)
    fpad[:n] = feats_sel
    lpad = np.zeros((B_CAP,), dtype=np.float32)
    lpad[:n] = labs_sel
    mpad = np.zeros((B_CAP,), dtype=np.float32)
    mpad[:n] = msk_sel

    in_maps = []
    for c in range(N_CORES):
        sl = slice(c * B_CORE, (c + 1) * B_CORE)
        fcore = fpad[sl]  # [2304, 768]
        # [ch, p, w, j, ks, b] <- [(ch w j b), (ks p)]
        ft_host = np.ascontiguousarray(
            fcore.reshape(FT_CHUNKS, PAIRS_PER_CHUNK, 2, 128, KS, 128)
            .transpose(0, 5, 1, 2, 4, 3)
            .astype(f8np)
        )
        lab_host = np.ascontiguousarray(
            lpad[sl].reshape(NT, 128).T.astype(bfnp)
        )
        msk_host = np.ascontiguousarray(mpad[sl].reshape(NT, 128).T)
        in_maps.append(
            {
                "ft": ft_host,
                "cnt": cnt_host,
                "iota": iota_host,
                "lab": lab_host,
                "msk": msk_host,
            }
        )

    kwargs = {}
    if _TRACE:
        kwargs["trace"] = True
        if _TRACE_DIR:
            kwargs["tmpdir"] = _TRACE_DIR
    res = run_bass_kernel_spmd(nc, in_maps, core_ids=list(range(N_CORES)), **kwargs)
    last_results = res

    return sum(float(res.results[c]["out"][0, 0]) for c in range(N_CORES))


def kernel(features, centers, labels, labelled_or_not):
    feats = np.asarray(features, dtype=np.float32)
    cents = np.asarray(centers, dtype=np.float32)
    labs = np.asarray(labels).astype(np.float32)
    lmask = np.asarray(labelled_or_not).astype(bool)

    # normalized (+ fp8-range-scaled), zero-padded, transposed centers
    cn = cents / np.maximum(
        np.linalg.norm(cents, axis=1, keepdims=True), EPS_COS
    )
    cn_pad = np.zeros((CPAD, D), dtype=np.float32)
    cn_pad[:C] = cn * CSCALE
    f8np = ml_dtypes.float8_e4m3
    bfnp = ml_dtypes.bfloat16
    cnt_host = np.ascontiguousarray(
        cn_pad.reshape(CPAD, KS, 128).transpose(2, 1, 0).astype(f8np)
    )
    iota_host = np.ascontiguousarray(
        np.broadcast_to(np.arange(CPAD, dtype=np.float32), (128, CPAD)).astype(bfnp)
    )

    idx = np.flatnonzero(lmask)
    total = 0.0
    # one pass in practice; the loop is correctness insurance for inputs
    # with more than B_CAP labelled rows
    for start in range(0, idx.size, B_CAP):
        sel = idx[start : start + B_CAP]
        total += _run_pass(
            feats[sel], labs[sel], np.ones(sel.size, np.float32),
            cnt_host, iota_host,
        )
    return np.array(-total, dtype=np.float32)
